# revision 11
# baseline (speedup 1.0000x reference)
"""Trainium2 Bass kernel for nn_BasenetFgnnMeanfield (factor-graph GNN message passing).

Math (per batch element, vf = node_feats[b] in [120, 128]):
    hf = mean(vf[h_cord], axis=1)                    # [105, 128]
    gf = mean(vf[g_cord], axis=1)                    # [455, 128]
    nff = concat([vf, hf, gf])                       # [680, 128]
    msg[n, d] = relu(concat(nff[n], nff[graph[n,d]]) @ W_edge + b_edge)
    out[n] = mean_d msg[n, d]                        # [680, 16]

Algebraic restructuring used here:
  * The edge MLP is linear before the relu, so
        concat(self, nbr) @ W = self @ W_self + nbr @ W_nbr.
  * hf/gf/nff are linear in vf, so project vf down to 16 dims FIRST:
        P = vf @ [W_self | W_nbr]   ([120, 32] per batch), then
        A = S @ P_self, B = S @ P_nbr   with S the [680, 120] averaging
    matrix (identity on variable rows, 1/3-weighted 3-hot on factor rows).
  * relu is positively homogeneous, so the /14 edge mean is folded into W
    and b on the host, and the padded duplicate edges of factor rows
    (their 3rd neighbor repeats 12x) become out = r0 + r1 + 12*r2.
  * b_edge is folded into A via P_self += b (rows of S sum to 1).

Data-parallel over batch: 8 cores x 8 batches. On-chip layout is
node-major: [128 partitions = node chunk, 128 free = (batch, e)], so the
graph-neighbor gather B[graph[n, d]] is a row gather, done by writing B to
a DRAM scratch and issuing two GPSIMD dma_gather ops (compile-time index
list, 128-row-aligned blocks; single_packet=False and <=3072 idxs per op —
larger single ops crash this runtime).
"""

import sys
from itertools import combinations

import numpy as np

sys.path.insert(0, "/opt/trn_rl_repo")

import concourse.bass as bass  # noqa: E402
import concourse.tile as tile  # noqa: E402
from concourse import library_config, masks, mybir  # noqa: E402
from concourse.bass_utils import run_bass_kernel_spmd  # noqa: E402
from concourse.library_overlay import lower_extended_insts  # noqa: E402

# ---------------------------------------------------------------- constants
N = 15
D = 128
E = 16
CN2 = N * (N - 1) // 2            # 105
CN3 = N * (N - 1) * (N - 2) // 6  # 455
NV = N + CN2                      # 120 variable nodes
NG = NV + CN2 + CN3               # 680 graph nodes
DEG = 14
B = 64
NCORES = 8
BPC = B // NCORES                 # 8 batches per core
BE = BPC * E                      # 128 = (batch, e) free width
NCH = 6                           # node chunks of 128 (6*128 = 768 >= 680)
NPAD = NCH * 128                  # 768 rows in the padded node dim

# gather slot list: blocks d=0,1,2 over all 680 rows (each padded to 768 so
# chunks align with node chunks), then d=3..13 over rows<120 (padded to 128)
NIDX = 3 * NPAD + (DEG - 3) * 128  # 3712
GCH = NIDX // 128                  # 29 gathered chunks
SPLIT = 1920                       # two dma_gathers: 1920 + 1792 idxs


def _build_structure():
    h_id = {c: i + N for i, c in enumerate(combinations(range(N), 2))}
    G = [[] for _ in range(N + N * (N - 1) + CN3)]
    hidx = N + CN2
    for u in range(N):
        for v in range(u + 1, N):
            G[hidx].extend([u, v, h_id[(u, v)]])
            G[u].append(hidx)
            G[v].append(hidx)
            G[h_id[(u, v)]].append(hidx)
            hidx += 1
    gidx = N + N * (N - 1)
    for i in range(N):
        for j in range(i + 1, N):
            for k in range(j + 1, N):
                z1, z2, z3 = h_id[(i, j)], h_id[(i, k)], h_id[(j, k)]
                G[gidx].extend([z1, z2, z3])
                G[z1].append(gidx)
                G[z2].append(gidx)
                G[z3].append(gidx)
                gidx += 1
    deg = max(3, N - 1)
    for l in G:
        while len(l) < deg:
            l.append(l[-1])
    graph = np.array(G, dtype=np.int32)  # [680, 14]
    h_cord = np.array(
        [[u, v, i + N] for i, (u, v) in enumerate(combinations(range(N), 2))],
        dtype=np.int32,
    )
    g_cord = np.array(
        [
            [h_id[(i, j)], h_id[(i, k)], h_id[(j, k)]]
            for i in range(N)
            for j in range(i + 1, N)
            for k in range(j + 1, N)
        ],
        dtype=np.int32,
    )
    return graph, h_cord, g_cord


def _host_tables():
    graph, h_cord, g_cord = _build_structure()

    # S^T [120, 680]: column n averages its member variable nodes.
    st = np.zeros((NV, NG), dtype=np.float32)
    st[np.arange(NV), np.arange(NV)] = 1.0
    for i, mem in enumerate(h_cord):
        st[mem, NV + i] = 1.0 / 3.0
    for i, mem in enumerate(g_cord):
        st[mem, NV + CN2 + i] = 1.0 / 3.0

    # gather slot list. Pad slots of blocks 0..2 gather row 0 (outputs unused).
    # Pad rows 120..127 of blocks 3..13 repeat block 2's gather (graph[n, 2]),
    # which makes the 11-block sum equal 11*r2 on those rows — exactly the
    # factor-row formula — so chunk 0 needs no partition-sliced special case
    # (SBUF partition offsets must be multiples of 32 anyway).
    L = np.zeros(NIDX, dtype=np.int16)
    for d in range(3):
        L[d * NPAD : d * NPAD + NG] = graph[:, d]
    for d in range(3, DEG):
        o = 3 * NPAD + (d - 3) * 128
        L[o : o + NV] = graph[:NV, d]
        L[o + NV : o + 128] = graph[NV:128, 2]

    # dma_gather wrapped layout: idx i lives at [i % 16, i // 16], same table
    # replicated across the 8 GPSIMD cores' 16-partition groups
    def wrap(lst):
        n = lst.shape[0]
        t = np.zeros((128, n // 16), dtype=np.int16)
        lanes = lst.reshape(n // 16, 16).T
        for c in range(8):
            t[16 * c : 16 * (c + 1)] = lanes
        return t

    return st, wrap(L[:SPLIT]), wrap(L[SPLIT:])


def _split_multi_waits(nc, max_waits=1):
    """Split instructions carrying multiple sem waits into EVSEM + inst.

    The walrus build here rejects instructions with more than one sync wait
    ("Too many sync wait commands"), while Tile's semaphore assignment
    freely attaches several. Hoist the extras onto standalone EventSemaphore
    instructions on the same engine, placed immediately before, which is
    semantically identical (the sequencer blocks on each in program order).
    """
    n = 0
    for fn in nc.m.functions:
        for bb in fn.blocks:
            out = []
            changed = False
            for inst in bb.instructions:
                si = inst.sync_info
                if si is not None and si.on_wait and len(si.on_wait) > max_waits:
                    waits = list(si.on_wait)
                    for w in waits[:-max_waits]:
                        n += 1
                        out.append(
                            mybir.InstEventSemaphore(
                                name=f"evsplit-{n}",
                                engine=inst.engine,
                                ins=[],
                                outs=[],
                                sync_info=mybir.SyncInfo(on_wait=[w], on_update=[]),
                            )
                        )
                    si.on_wait = waits[-max_waits:]
                    changed = True
                out.append(inst)
            if changed:
                bb.instructions = out


def _build_bass():
    st_np, gi1_np, gi2_np = _host_tables()
    f32 = mybir.dt.float32

    nc = bass.Bass("TRN2")
    x_dram = nc.dram_tensor("x", [BPC, NV, D], f32, kind="ExternalInput")
    w_dram = nc.dram_tensor("w", [D, 2, E], f32, kind="ExternalInput")
    b_dram = nc.dram_tensor("bvec", [BE], f32, kind="ExternalInput")
    out_dram = nc.dram_tensor("out", [BPC, NG, E], f32, kind="ExternalOutput")
    bscratch = nc.dram_tensor("bscratch", [NPAD, BE], f32)
    st_dram = nc.inline_tensor(st_np, name="st_const")
    gi1_dram = nc.inline_tensor(gi1_np, name="gi1_const")
    gi2_dram = nc.inline_tensor(gi2_np, name="gi2_const")

    with tile.TileContext(nc) as tc:
        with (
            tc.tile_pool(name="consts", bufs=1) as consts,
            tc.tile_pool(name="work", bufs=1) as work,
            tc.tile_pool(name="ps_stage1", bufs=2, space="PSUM") as ps1,
            tc.tile_pool(name="ps_ab", bufs=4, space="PSUM") as psab,
        ):
            # ---- constant / input loads
            ident = consts.tile([128, 128], f32)
            masks.make_identity(nc, ident)
            w_sb = consts.tile([128, 2, E], f32)
            nc.sync.dma_start(out=w_sb, in_=w_dram[:, :, :])
            st_sb = consts.tile([NV, NG], f32)
            nc.sync.dma_start(out=st_sb, in_=st_dram[:, :])
            gi1_sb = consts.tile([128, SPLIT // 16], mybir.dt.int16)
            nc.sync.dma_start(out=gi1_sb, in_=gi1_dram[:, :])
            gi2_sb = consts.tile([128, (NIDX - SPLIT) // 16], mybir.dt.int16)
            nc.sync.dma_start(out=gi2_sb, in_=gi2_dram[:, :])
            b2p_sb = consts.tile([NV, BE], f32)
            nc.sync.dma_start(
                out=b2p_sb,
                in_=bass.AP(
                    tensor=b_dram, offset=0, ap=[[0, NV], [1, BE]]
                ),
            )
            x_sb = work.tile([NV, BPC, D], f32)
            for b in range(BPC):
                nc.sync.dma_start(out=x_sb[:, b, :], in_=x_dram[b, :, :])

            libload = nc.gpsimd.load_library(library_config.mlp)

            # ---- stage 1: per batch, transpose vf and project to 32 dims
            p_sb = work.tile([NV, 2, BPC, E], f32)  # (half, batch, e)
            for b in range(BPC):
                xt_ps = ps1.tile([D, NV], f32, tag="xt")
                nc.tensor.transpose(xt_ps, x_sb[:, b, :], ident[:NV, :NV])
                xt_sb = work.tile([D, NV], f32, tag="xt_sb")
                nc.vector.tensor_copy(xt_sb, xt_ps)
                pp = ps1.tile([NV, 2, E], f32, tag="pp")
                nc.tensor.matmul(
                    pp.rearrange("p h e -> p (h e)"),
                    lhsT=xt_sb,
                    rhs=w_sb.rearrange("p h e -> p (h e)"),
                    start=True,
                    stop=True,
                )
                nc.scalar.copy(out=p_sb[:, :, b, :], in_=pp)
            # fold b_edge into A: rows of S sum to 1, so S@(P_self + b) = A + b
            nc.vector.tensor_add(p_sb[:, 0, :, :], p_sb[:, 0, :, :], b2p_sb)

            # ---- stage 2: A = S@P_self (+b), B = S@P_nbr, node-major
            a_sb = work.tile([128, NCH, BE], f32)
            b_sb = work.tile([128, NCH, BE], f32)
            for c in range(NCH):
                n0 = 128 * c
                nw = min(128, NG - n0)
                for h, dst in ((0, a_sb), (1, b_sb)):
                    ps = psab.tile([128, BE], f32, tag="ab")
                    nc.tensor.matmul(
                        ps[:nw, :],
                        lhsT=st_sb[:, n0 : n0 + nw],
                        rhs=p_sb[:, h, :, :].rearrange("p b e -> p (b e)"),
                        start=True,
                        stop=True,
                    )
                    nc.vector.tensor_copy(dst[:nw, c, :], ps[:nw, :])
            bdma = nc.sync.dma_start(
                out=bscratch[:, :].rearrange("(c p) e -> p c e", p=128),
                in_=b_sb[:, :, :],
            )

            # ---- stage 3: gather B rows over the graph neighbor table
            g_sb = work.tile([128, GCH, BE], f32)
            g1 = nc.gpsimd.dma_gather(
                out_ap=g_sb[:, : SPLIT // 128, :],
                in_ap=bscratch[:, :],
                idxs_ap=gi1_sb[:, :],
                num_idxs=SPLIT,
                num_idxs_reg=SPLIT,
                elem_size=BE,
                single_packet=False,
            )
            g2 = nc.gpsimd.dma_gather(
                out_ap=g_sb[:, SPLIT // 128 :, :],
                in_ap=bscratch[:, :],
                idxs_ap=gi2_sb[:, :],
                num_idxs=NIDX - SPLIT,
                num_idxs_reg=NIDX - SPLIT,
                elem_size=BE,
                single_packet=False,
            )
            # library load must precede the gathers (add_dep_helper(a, b) = a
            # waits on b); the bscratch RAW dep is tracked by Tile itself
            del bdma
            tile.add_dep_helper(g1.ins, libload.ins, reason="gather after lib load")
            tile.add_dep_helper(g2.ins, libload.ins, reason="gather after lib load")

            # ---- stage 4: r = relu(A + G); weighted sum over the 14 slots
            r_sb = work.tile([128, GCH, BE], f32)
            for k in range(3):
                nc.vector.tensor_add(
                    r_sb[:, 6 * k : 6 * (k + 1), :],
                    g_sb[:, 6 * k : 6 * (k + 1), :],
                    a_sb,
                )
            a0 = a_sb[:, 0, :]
            a0_b11 = bass.AP(
                tensor=a0.tensor,
                offset=a0.offset,
                ap=[a0.ap[0], [0, DEG - 3], a0.ap[-1]],
            )
            nc.vector.tensor_add(r_sb[:, 18:, :], g_sb[:, 18:, :], a0_b11)
            half = GCH // 2
            nc.scalar.activation(
                r_sb[:, :half, :],
                r_sb[:, :half, :],
                mybir.ActivationFunctionType.Relu,
            )
            nc.vector.tensor_scalar_max(r_sb[:, half:, :], r_sb[:, half:, :], 0.0)

            out_sb = work.tile([128, NCH, BE], f32)
            s_sb = work.tile([128, NCH, BE], f32)
            nc.vector.tensor_add(s_sb, r_sb[:, 0:6, :], r_sb[:, 6:12, :])
            nc.vector.tensor_add(s_sb, s_sb, r_sb[:, 12:18, :])
            # factor rows (chunks 1..5): the third neighbor appears 12x in
            # the padded table -> add 11*r2 more
            nc.vector.scalar_tensor_tensor(
                out_sb[:, 1:, :],
                r_sb[:, 13:18, :],
                11.0,
                s_sb[:, 1:, :],
                op0=mybir.AluOpType.mult,
                op1=mybir.AluOpType.add,
            )
            # chunk 0: rows <120 sum neighbor slots 3..13; rows 120..127 get
            # 11*r2 via the index-table trick (see _host_tables)
            t1 = work.tile([128, 5, BE], f32)
            nc.vector.tensor_add(t1, r_sb[:, 18:23, :], r_sb[:, 23:28, :])
            t3 = work.tile([128, BE], f32)
            nc.vector.tensor_add(t3, t1[:, 0, :], t1[:, 1, :])
            nc.vector.tensor_add(t3, t3, t1[:, 2, :])
            nc.vector.tensor_add(t3, t3, t1[:, 3, :])
            nc.vector.tensor_add(t3, t3, t1[:, 4, :])
            nc.vector.tensor_add(t3, t3, r_sb[:, 28, :])
            nc.vector.tensor_add(out_sb[:, 0, :], s_sb[:, 0, :], t3)

            # ---- stage 5: store (already node-major, e contiguous per batch)
            for b in range(BPC):
                dst = out_dram[b, 0:640, :].rearrange("(c p) e -> p c e", p=128)
                nc.sync.dma_start(
                    out=dst, in_=out_sb[:, 0:5, E * b : E * (b + 1)]
                )
                nc.sync.dma_start(
                    out=out_dram[b, 640:NG, :],
                    in_=out_sb[0 : NG - 640, 5, E * b : E * (b + 1)],
                )
    _split_multi_waits(nc)
    lower_extended_insts(nc)
    return nc


_NC_CACHE = None


def _get_nc():
    global _NC_CACHE
    if _NC_CACHE is None:
        _NC_CACHE = _build_bass()
    return _NC_CACHE


def _in_maps(node_feats, W_edge, b_edge):
    # fold the /14 edge mean into the MLP weights (relu is pos. homogeneous)
    ws = (W_edge / DEG).reshape(2, D, E).transpose(1, 0, 2)  # [128, 2, 16]
    ws = np.ascontiguousarray(ws.astype(np.float32))
    bvec = np.ascontiguousarray(
        np.tile((b_edge / DEG).astype(np.float32), BPC)
    )  # [(b, e)] = 128
    return [
        {
            "x": np.ascontiguousarray(
                node_feats[c * BPC : (c + 1) * BPC].astype(np.float32)
            ),
            "w": ws,
            "bvec": bvec,
        }
        for c in range(NCORES)
    ]


def kernel(node_feats, W_edge, b_edge):
    node_feats = np.asarray(node_feats, dtype=np.float32)
    W_edge = np.asarray(W_edge, dtype=np.float32)
    b_edge = np.asarray(b_edge, dtype=np.float32)
    assert node_feats.shape == (B, NV, D)

    nc = _get_nc()
    res = run_bass_kernel_spmd(
        nc, _in_maps(node_feats, W_edge, b_edge), core_ids=list(range(NCORES))
    )
    out = np.concatenate([res.results[c]["out"] for c in range(NCORES)], axis=0)
    return out


# revision 13
# speedup vs baseline: 1.1284x; 1.1284x over previous
"""Trainium2 Bass kernel for nn_BasenetFgnnMeanfield (factor-graph GNN message passing).

Math (per batch element, vf = node_feats[b] in [120, 128]):
    hf = mean(vf[h_cord], axis=1)                    # [105, 128]
    gf = mean(vf[g_cord], axis=1)                    # [455, 128]
    nff = concat([vf, hf, gf])                       # [680, 128]
    msg[n, d] = relu(concat(nff[n], nff[graph[n,d]]) @ W_edge + b_edge)
    out[n] = mean_d msg[n, d]                        # [680, 16]

Algebraic restructuring used here:
  * The edge MLP is linear before the relu, so
        concat(self, nbr) @ W = self @ W_self + nbr @ W_nbr.
  * hf/gf/nff are linear in vf, so project vf down to 16 dims FIRST:
        P = vf @ [W_self | W_nbr]   ([120, 32] per batch), then
        A = S @ P_self, B = S @ P_nbr   with S the [680, 120] averaging
    matrix (identity on variable rows, 1/3-weighted 3-hot on factor rows).
  * relu is positively homogeneous, so the /14 edge mean is folded into W
    and b on the host, and the padded duplicate edges of factor rows
    (their 3rd neighbor repeats 12x) become out = r0 + r1 + 12*r2.
  * b_edge is folded into A via P_self += b (rows of S sum to 1).

Data-parallel over batch: 8 cores x 8 batches. On-chip layout is
node-major: [128 partitions = node chunk, 128 free = (batch, e)], so the
graph-neighbor gather B[graph[n, d]] is a row gather, done by writing B to
a DRAM scratch and issuing two GPSIMD dma_gather ops (compile-time index
list, 128-row-aligned blocks; single_packet=False and <=3072 idxs per op —
larger single ops crash this runtime).
"""

import sys
from itertools import combinations

import numpy as np

sys.path.insert(0, "/opt/trn_rl_repo")

import concourse.bass as bass  # noqa: E402
import concourse.tile as tile  # noqa: E402
from concourse import library_config, mybir  # noqa: E402
from concourse.bass_utils import run_bass_kernel_spmd  # noqa: E402
from concourse.library_overlay import lower_extended_insts  # noqa: E402

# ---------------------------------------------------------------- constants
N = 15
D = 128
E = 16
CN2 = N * (N - 1) // 2            # 105
CN3 = N * (N - 1) * (N - 2) // 6  # 455
NV = N + CN2                      # 120 variable nodes
NG = NV + CN2 + CN3               # 680 graph nodes
DEG = 14
B = 64
NCORES = 8
BPC = B // NCORES                 # 8 batches per core
BE = BPC * E                      # 128 = (batch, e) free width
NCH = 6                           # node chunks of 128 (6*128 = 768 >= 680)
NPAD = NCH * 128                  # 768 rows in the padded node dim

# gather slot list: blocks d=0,1,2 over all 680 rows (each padded to 768 so
# chunks align with node chunks), then d=3..13 over rows<120 (padded to 128)
NIDX = 3 * NPAD + (DEG - 3) * 128  # 3712
GCH = NIDX // 128                  # 29 gathered chunks
SPLIT = 1920                       # two dma_gathers: 1920 + 1792 idxs


def _build_structure():
    h_id = {c: i + N for i, c in enumerate(combinations(range(N), 2))}
    G = [[] for _ in range(N + N * (N - 1) + CN3)]
    hidx = N + CN2
    for u in range(N):
        for v in range(u + 1, N):
            G[hidx].extend([u, v, h_id[(u, v)]])
            G[u].append(hidx)
            G[v].append(hidx)
            G[h_id[(u, v)]].append(hidx)
            hidx += 1
    gidx = N + N * (N - 1)
    for i in range(N):
        for j in range(i + 1, N):
            for k in range(j + 1, N):
                z1, z2, z3 = h_id[(i, j)], h_id[(i, k)], h_id[(j, k)]
                G[gidx].extend([z1, z2, z3])
                G[z1].append(gidx)
                G[z2].append(gidx)
                G[z3].append(gidx)
                gidx += 1
    deg = max(3, N - 1)
    for l in G:
        while len(l) < deg:
            l.append(l[-1])
    graph = np.array(G, dtype=np.int32)  # [680, 14]
    h_cord = np.array(
        [[u, v, i + N] for i, (u, v) in enumerate(combinations(range(N), 2))],
        dtype=np.int32,
    )
    g_cord = np.array(
        [
            [h_id[(i, j)], h_id[(i, k)], h_id[(j, k)]]
            for i in range(N)
            for j in range(i + 1, N)
            for k in range(j + 1, N)
        ],
        dtype=np.int32,
    )
    return graph, h_cord, g_cord


def _host_tables():
    graph, h_cord, g_cord = _build_structure()

    # S^T [120, 680]: column n averages its member variable nodes.
    st = np.zeros((NV, NG), dtype=np.float32)
    st[np.arange(NV), np.arange(NV)] = 1.0
    for i, mem in enumerate(h_cord):
        st[mem, NV + i] = 1.0 / 3.0
    for i, mem in enumerate(g_cord):
        st[mem, NV + CN2 + i] = 1.0 / 3.0

    # gather slot list. Pad slots of blocks 0..2 gather row 0 (outputs unused).
    # Pad rows 120..127 of blocks 3..13 repeat block 2's gather (graph[n, 2]),
    # which makes the 11-block sum equal 11*r2 on those rows — exactly the
    # factor-row formula — so chunk 0 needs no partition-sliced special case
    # (SBUF partition offsets must be multiples of 32 anyway).
    L = np.zeros(NIDX, dtype=np.int16)
    for d in range(3):
        L[d * NPAD : d * NPAD + NG] = graph[:, d]
    for d in range(3, DEG):
        o = 3 * NPAD + (d - 3) * 128
        L[o : o + NV] = graph[:NV, d]
        L[o + NV : o + 128] = graph[NV:128, 2]

    # dma_gather wrapped layout: idx i lives at [i % 16, i // 16], same table
    # replicated across the 8 GPSIMD cores' 16-partition groups
    def wrap(lst):
        n = lst.shape[0]
        t = np.zeros((128, n // 16), dtype=np.int16)
        lanes = lst.reshape(n // 16, 16).T
        for c in range(8):
            t[16 * c : 16 * (c + 1)] = lanes
        return t

    return st, wrap(L[:SPLIT]), wrap(L[SPLIT:])


def _split_multi_waits(nc, max_waits=1):
    """Split instructions carrying multiple sem waits into EVSEM + inst.

    The walrus build here rejects instructions with more than one sync wait
    ("Too many sync wait commands"), while Tile's semaphore assignment
    freely attaches several. Hoist the extras onto standalone EventSemaphore
    instructions on the same engine, placed immediately before, which is
    semantically identical (the sequencer blocks on each in program order).
    """
    n = 0
    for fn in nc.m.functions:
        for bb in fn.blocks:
            out = []
            changed = False
            for inst in bb.instructions:
                si = inst.sync_info
                if si is not None and si.on_wait and len(si.on_wait) > max_waits:
                    waits = list(si.on_wait)
                    for w in waits[:-max_waits]:
                        n += 1
                        out.append(
                            mybir.InstEventSemaphore(
                                name=f"evsplit-{n}",
                                engine=inst.engine,
                                ins=[],
                                outs=[],
                                sync_info=mybir.SyncInfo(on_wait=[w], on_update=[]),
                            )
                        )
                    si.on_wait = waits[-max_waits:]
                    changed = True
                out.append(inst)
            if changed:
                bb.instructions = out


def _build_bass():
    st_np, gi1_np, gi2_np = _host_tables()
    f32 = mybir.dt.float32

    nc = bass.Bass("TRN2", num_swdge_queues=2)
    x_dram = nc.dram_tensor("x", [BPC, NV, D], f32, kind="ExternalInput")
    w_dram = nc.dram_tensor("w", [D, 2, E], f32, kind="ExternalInput")
    b_dram = nc.dram_tensor("bvec", [BE], f32, kind="ExternalInput")
    out_dram = nc.dram_tensor("out", [BPC, NG, E], f32, kind="ExternalOutput")
    bscratch = nc.dram_tensor("bscratch", [NPAD, BE], f32)
    st_dram = nc.inline_tensor(st_np, name="st_const")
    id_dram = nc.inline_tensor(np.eye(128, dtype=np.float32), name="id_const")
    gi1_dram = nc.inline_tensor(gi1_np, name="gi1_const")
    gi2_dram = nc.inline_tensor(gi2_np, name="gi2_const")

    with tile.TileContext(nc) as tc:
        with (
            tc.tile_pool(name="consts", bufs=1) as consts,
            tc.tile_pool(name="work", bufs=1) as work,
            tc.tile_pool(name="ps_stage1", bufs=2, space="PSUM") as ps1,
            tc.tile_pool(name="ps_ab", bufs=4, space="PSUM") as psab,
        ):
            # ---- constant / input loads
            ident = consts.tile([128, 128], f32)
            nc.sync.dma_start(out=ident, in_=id_dram[:, :])
            w_sb = consts.tile([128, 2, E], f32)
            nc.sync.dma_start(out=w_sb, in_=w_dram[:, :, :])
            st_sb = consts.tile([NV, NG], f32)
            nc.sync.dma_start(out=st_sb, in_=st_dram[:, :])
            gi1_sb = consts.tile([128, SPLIT // 16], mybir.dt.int16)
            nc.sync.dma_start(out=gi1_sb, in_=gi1_dram[:, :])
            gi2_sb = consts.tile([128, (NIDX - SPLIT) // 16], mybir.dt.int16)
            nc.sync.dma_start(out=gi2_sb, in_=gi2_dram[:, :])
            b2p_sb = consts.tile([NV, BE], f32)
            nc.sync.dma_start(
                out=b2p_sb,
                in_=bass.AP(
                    tensor=b_dram, offset=0, ap=[[0, NV], [1, BE]]
                ),
            )
            x_sb = work.tile([NV, BPC, D], f32)
            nc.sync.dma_start(
                out=x_sb[:, :, :],
                in_=bass.AP(
                    tensor=x_dram,
                    offset=0,
                    ap=[[D, NV], [NV * D, BPC], [1, D]],
                ),
            )

            libload = nc.gpsimd.load_library(library_config.mlp)

            # ---- stage 1: per batch, transpose vf and project to 32 dims
            p_sb = work.tile([NV, 2, BPC, E], f32)  # (half, batch, e)
            for b in range(BPC):
                xt_ps = ps1.tile([D, NV], f32, tag="xt")
                nc.tensor.transpose(xt_ps, x_sb[:, b, :], ident[:NV, :NV])
                xt_sb = work.tile([D, NV], f32, tag="xt_sb")
                nc.vector.tensor_copy(xt_sb, xt_ps)
                pp = ps1.tile([NV, 2, E], f32, tag="pp")
                nc.tensor.matmul(
                    pp.rearrange("p h e -> p (h e)"),
                    lhsT=xt_sb,
                    rhs=w_sb.rearrange("p h e -> p (h e)"),
                    start=True,
                    stop=True,
                )
                nc.scalar.copy(out=p_sb[:, :, b, :], in_=pp)
            # fold b_edge into A: rows of S sum to 1, so S@(P_self + b) = A + b
            nc.vector.tensor_add(p_sb[:, 0, :, :], p_sb[:, 0, :, :], b2p_sb)

            # ---- stage 2: A = S@P_self (+b), B = S@P_nbr, node-major
            a_sb = work.tile([128, NCH, BE], f32)
            b_sb = work.tile([128, NCH, BE], f32)
            for c in range(NCH):
                n0 = 128 * c
                nw = min(128, NG - n0)
                for h, dst in ((0, a_sb), (1, b_sb)):
                    ps = psab.tile([128, BE], f32, tag="ab")
                    nc.tensor.matmul(
                        ps[:nw, :],
                        lhsT=st_sb[:, n0 : n0 + nw],
                        rhs=p_sb[:, h, :, :].rearrange("p b e -> p (b e)"),
                        start=True,
                        stop=True,
                    )
                    nc.vector.tensor_copy(dst[:nw, c, :], ps[:nw, :])
            bdma = nc.sync.dma_start(
                out=bscratch[:, :].rearrange("(c p) e -> p c e", p=128),
                in_=b_sb[:, :, :],
            )

            # ---- stage 3: gather B rows over the graph neighbor table
            g_sb = work.tile([128, GCH, BE], f32)
            g1 = nc.gpsimd.dma_gather(
                out_ap=g_sb[:, : SPLIT // 128, :],
                in_ap=bscratch[:, :],
                idxs_ap=gi1_sb[:, :],
                num_idxs=SPLIT,
                num_idxs_reg=SPLIT,
                elem_size=BE,
                single_packet=False,
                queue_num=0,
            )
            g2 = nc.gpsimd.dma_gather(
                out_ap=g_sb[:, SPLIT // 128 :, :],
                in_ap=bscratch[:, :],
                idxs_ap=gi2_sb[:, :],
                num_idxs=NIDX - SPLIT,
                num_idxs_reg=NIDX - SPLIT,
                elem_size=BE,
                single_packet=False,
                queue_num=1,
            )
            # library load must precede the gathers (add_dep_helper(a, b) = a
            # waits on b); the bscratch RAW dep is tracked by Tile itself
            del bdma
            tile.add_dep_helper(g1.ins, libload.ins, reason="gather after lib load")
            tile.add_dep_helper(g2.ins, libload.ins, reason="gather after lib load")

            # ---- stage 4: r = relu(A + G); weighted sum over the 14 slots
            r_sb = work.tile([128, GCH, BE], f32)
            for k in range(3):
                nc.vector.tensor_add(
                    r_sb[:, 6 * k : 6 * (k + 1), :],
                    g_sb[:, 6 * k : 6 * (k + 1), :],
                    a_sb,
                )
            a0 = a_sb[:, 0, :]
            a0_b11 = bass.AP(
                tensor=a0.tensor,
                offset=a0.offset,
                ap=[a0.ap[0], [0, DEG - 3], a0.ap[-1]],
            )
            nc.vector.tensor_add(r_sb[:, 18:, :], g_sb[:, 18:, :], a0_b11)
            half = GCH // 2
            nc.scalar.activation(
                r_sb[:, :half, :],
                r_sb[:, :half, :],
                mybir.ActivationFunctionType.Relu,
            )
            nc.vector.tensor_scalar_max(r_sb[:, half:, :], r_sb[:, half:, :], 0.0)

            out_sb = work.tile([128, NCH, BE], f32)
            s_sb = work.tile([128, NCH, BE], f32)
            nc.vector.tensor_add(s_sb, r_sb[:, 0:6, :], r_sb[:, 6:12, :])
            nc.vector.tensor_add(s_sb, s_sb, r_sb[:, 12:18, :])
            # factor rows (chunks 1..5): the third neighbor appears 12x in
            # the padded table -> add 11*r2 more
            nc.vector.scalar_tensor_tensor(
                out_sb[:, 1:, :],
                r_sb[:, 13:18, :],
                11.0,
                s_sb[:, 1:, :],
                op0=mybir.AluOpType.mult,
                op1=mybir.AluOpType.add,
            )
            # chunk 0: rows <120 sum neighbor slots 3..13; rows 120..127 get
            # 11*r2 via the index-table trick (see _host_tables)
            t1 = work.tile([128, 5, BE], f32)
            nc.vector.tensor_add(t1, r_sb[:, 18:23, :], r_sb[:, 23:28, :])
            t3 = work.tile([128, BE], f32)
            nc.vector.tensor_add(t3, t1[:, 0, :], t1[:, 1, :])
            nc.vector.tensor_add(t3, t3, t1[:, 2, :])
            nc.vector.tensor_add(t3, t3, t1[:, 3, :])
            nc.vector.tensor_add(t3, t3, t1[:, 4, :])
            nc.vector.tensor_add(t3, t3, r_sb[:, 28, :])
            nc.vector.tensor_add(out_sb[:, 0, :], s_sb[:, 0, :], t3)

            # ---- stage 5: store (already node-major, e contiguous per batch)
            for b in range(BPC):
                dst = out_dram[b, 0:640, :].rearrange("(c p) e -> p c e", p=128)
                nc.sync.dma_start(
                    out=dst, in_=out_sb[:, 0:5, E * b : E * (b + 1)]
                )
                nc.sync.dma_start(
                    out=out_dram[b, 640:NG, :],
                    in_=out_sb[0 : NG - 640, 5, E * b : E * (b + 1)],
                )
    _split_multi_waits(nc)
    lower_extended_insts(nc)
    return nc


_NC_CACHE = None


def _get_nc():
    global _NC_CACHE
    if _NC_CACHE is None:
        _NC_CACHE = _build_bass()
    return _NC_CACHE


def _in_maps(node_feats, W_edge, b_edge):
    # fold the /14 edge mean into the MLP weights (relu is pos. homogeneous)
    ws = (W_edge / DEG).reshape(2, D, E).transpose(1, 0, 2)  # [128, 2, 16]
    ws = np.ascontiguousarray(ws.astype(np.float32))
    bvec = np.ascontiguousarray(
        np.tile((b_edge / DEG).astype(np.float32), BPC)
    )  # [(b, e)] = 128
    return [
        {
            "x": np.ascontiguousarray(
                node_feats[c * BPC : (c + 1) * BPC].astype(np.float32)
            ),
            "w": ws,
            "bvec": bvec,
        }
        for c in range(NCORES)
    ]


def kernel(node_feats, W_edge, b_edge):
    node_feats = np.asarray(node_feats, dtype=np.float32)
    W_edge = np.asarray(W_edge, dtype=np.float32)
    b_edge = np.asarray(b_edge, dtype=np.float32)
    assert node_feats.shape == (B, NV, D)

    nc = _get_nc()
    res = run_bass_kernel_spmd(
        nc, _in_maps(node_feats, W_edge, b_edge), core_ids=list(range(NCORES))
    )
    out = np.concatenate([res.results[c]["out"] for c in range(NCORES)], axis=0)
    return out


# revision 17
# speedup vs baseline: 1.3844x; 1.2269x over previous
"""Trainium2 Bass kernel for nn_BasenetFgnnMeanfield (factor-graph GNN message passing).

Math (per batch element, vf = node_feats[b] in [120, 128]):
    hf = mean(vf[h_cord], axis=1)                    # [105, 128]
    gf = mean(vf[g_cord], axis=1)                    # [455, 128]
    nff = concat([vf, hf, gf])                       # [680, 128]
    msg[n, d] = relu(concat(nff[n], nff[graph[n,d]]) @ W_edge + b_edge)
    out[n] = mean_d msg[n, d]                        # [680, 16]

Algebraic restructuring used here:
  * The edge MLP is linear before the relu, so
        concat(self, nbr) @ W = self @ W_self + nbr @ W_nbr.
  * hf/gf/nff are linear in vf, so project vf down to 16 dims FIRST:
        P = vf @ [W_self | W_nbr]   ([120, 32] per batch), then
        A = S @ P_self, B = S @ P_nbr   with S the [680, 120] averaging
    matrix (identity on variable rows, 1/3-weighted 3-hot on factor rows).
  * relu is positively homogeneous, so the /14 edge mean is folded into W
    and b on the host, and the padded duplicate edges of factor rows
    (their 3rd neighbor repeats 12x) become out = r0 + r1 + 12*r2.
  * b_edge is folded into A via P_self += b (rows of S sum to 1).

Data-parallel over batch: 8 cores x 8 batches. On-chip layout is
node-major: [128 partitions = node chunk, 128 free = (batch, e)], so the
graph-neighbor gather B[graph[n, d]] is a row gather, done by writing B to
a DRAM scratch and issuing two GPSIMD dma_gather ops (compile-time index
list, 128-row-aligned blocks; single_packet=False and <=3072 idxs per op —
larger single ops crash this runtime).
"""

import sys
from itertools import combinations

import numpy as np

sys.path.insert(0, "/opt/trn_rl_repo")

import concourse.bass as bass  # noqa: E402
import concourse.tile as tile  # noqa: E402
from concourse import library_config, mybir  # noqa: E402
from concourse.bass_utils import run_bass_kernel_spmd  # noqa: E402
from concourse.library_overlay import lower_extended_insts  # noqa: E402

# ---------------------------------------------------------------- constants
N = 15
D = 128
E = 16
CN2 = N * (N - 1) // 2            # 105
CN3 = N * (N - 1) * (N - 2) // 6  # 455
NV = N + CN2                      # 120 variable nodes
NG = NV + CN2 + CN3               # 680 graph nodes
DEG = 14
B = 64
NCORES = 8
BPC = B // NCORES                 # 8 batches per core
BE = BPC * E                      # 128 = (batch, e) free width
NCH = 6                           # node chunks of 128 (6*128 = 768 >= 680)
NPAD = NCH * 128                  # 768 rows in the padded node dim

# gather slot list: blocks d=0,1,2 over all 680 rows (each padded to 768 so
# chunks align with node chunks), then d=3..13 over rows<120 (padded to 128)
NIDX = 3 * NPAD + (DEG - 3) * 128  # 3712
GCH = NIDX // 128                  # 29 gathered chunks
SPLIT = 1920                       # two dma_gathers: 1920 + 1792 idxs


def _build_structure():
    h_id = {c: i + N for i, c in enumerate(combinations(range(N), 2))}
    G = [[] for _ in range(N + N * (N - 1) + CN3)]
    hidx = N + CN2
    for u in range(N):
        for v in range(u + 1, N):
            G[hidx].extend([u, v, h_id[(u, v)]])
            G[u].append(hidx)
            G[v].append(hidx)
            G[h_id[(u, v)]].append(hidx)
            hidx += 1
    gidx = N + N * (N - 1)
    for i in range(N):
        for j in range(i + 1, N):
            for k in range(j + 1, N):
                z1, z2, z3 = h_id[(i, j)], h_id[(i, k)], h_id[(j, k)]
                G[gidx].extend([z1, z2, z3])
                G[z1].append(gidx)
                G[z2].append(gidx)
                G[z3].append(gidx)
                gidx += 1
    deg = max(3, N - 1)
    for l in G:
        while len(l) < deg:
            l.append(l[-1])
    graph = np.array(G, dtype=np.int32)  # [680, 14]
    h_cord = np.array(
        [[u, v, i + N] for i, (u, v) in enumerate(combinations(range(N), 2))],
        dtype=np.int32,
    )
    g_cord = np.array(
        [
            [h_id[(i, j)], h_id[(i, k)], h_id[(j, k)]]
            for i in range(N)
            for j in range(i + 1, N)
            for k in range(j + 1, N)
        ],
        dtype=np.int32,
    )
    return graph, h_cord, g_cord


def _host_tables():
    graph, h_cord, g_cord = _build_structure()

    # S^T [120, 680]: column n averages its member variable nodes.
    st = np.zeros((NV, NG), dtype=np.float32)
    st[np.arange(NV), np.arange(NV)] = 1.0
    for i, mem in enumerate(h_cord):
        st[mem, NV + i] = 1.0 / 3.0
    for i, mem in enumerate(g_cord):
        st[mem, NV + CN2 + i] = 1.0 / 3.0

    # gather slot list. Pad slots of blocks 0..2 gather row 0 (outputs unused).
    # Pad rows 120..127 of blocks 3..13 repeat block 2's gather (graph[n, 2]),
    # which makes the 11-block sum equal 11*r2 on those rows — exactly the
    # factor-row formula — so chunk 0 needs no partition-sliced special case
    # (SBUF partition offsets must be multiples of 32 anyway).
    L = np.zeros(NIDX, dtype=np.int16)
    for d in range(3):
        L[d * NPAD : d * NPAD + NG] = graph[:, d]
    for d in range(3, DEG):
        o = 3 * NPAD + (d - 3) * 128
        L[o : o + NV] = graph[:NV, d]
        L[o + NV : o + 128] = graph[NV:128, 2]

    # dma_gather wrapped layout: idx i lives at [i % 16, i // 16], same table
    # replicated across the 8 GPSIMD cores' 16-partition groups
    def wrap(lst):
        n = lst.shape[0]
        t = np.zeros((128, n // 16), dtype=np.int16)
        lanes = lst.reshape(n // 16, 16).T
        for c in range(8):
            t[16 * c : 16 * (c + 1)] = lanes
        return t

    return st, wrap(L[:SPLIT]), wrap(L[SPLIT:])


def _split_multi_waits(nc, max_waits=1):
    """Split instructions carrying multiple sem waits into EVSEM + inst.

    The walrus build here rejects instructions with more than one sync wait
    ("Too many sync wait commands"), while Tile's semaphore assignment
    freely attaches several. Hoist the extras onto standalone EventSemaphore
    instructions on the same engine, placed immediately before, which is
    semantically identical (the sequencer blocks on each in program order).
    """
    n = 0
    for fn in nc.m.functions:
        for bb in fn.blocks:
            out = []
            changed = False
            for inst in bb.instructions:
                si = inst.sync_info
                if si is not None and si.on_wait and len(si.on_wait) > max_waits:
                    waits = list(si.on_wait)
                    for w in waits[:-max_waits]:
                        n += 1
                        out.append(
                            mybir.InstEventSemaphore(
                                name=f"evsplit-{n}",
                                engine=inst.engine,
                                ins=[],
                                outs=[],
                                sync_info=mybir.SyncInfo(on_wait=[w], on_update=[]),
                            )
                        )
                    si.on_wait = waits[-max_waits:]
                    changed = True
                out.append(inst)
            if changed:
                bb.instructions = out


def _fix_prep_sems(nc, lane_to_sem):
    """Point data-consumer waits at the prep DMA-completion sems.

    Tile books each prepare_only dma_gather on a DMASW lane and emits
    consumer waits on that lane's semaphore, but on hardware the gather's
    completion bumps the sem baked into its descriptors (the sem= arg).
    Rewrite lane waits to the descriptor sems. Lane waits emitted BEFORE the
    triggers are spurious WAR edges from the prep's deferred source read
    (the trigger already orders against the source write) — drop those.
    """
    for fn in nc.m.functions:
        for bb in fn.blocks:
            for inst in bb.instructions:
                si = inst.sync_info
                if not si or not si.on_wait:
                    continue
                is_bscratch_write = type(
                    inst
                ).__name__ == "InstDMACopy" and "bscratch" in str(inst.outs[:1])
                keep = []
                for w in si.on_wait:
                    lane = next(
                        (k for k in lane_to_sem if w.ant_name.startswith(k)), None
                    )
                    if lane is None:
                        keep.append(w)
                    elif not is_bscratch_write:
                        h = lane_to_sem[lane]
                        assert w.wait_value == 16, w
                        w.id = h.num
                        w.ant_name = h.name
                        keep.append(w)
                    # else: drop the spurious WAR wait on the source write
                si.on_wait = keep


def _build_bass():
    st_np, gi1_np, gi2_np = _host_tables()
    f32 = mybir.dt.float32

    nc = bass.Bass("TRN2", num_swdge_queues=2)
    x_dram = nc.dram_tensor("x", [BPC, NV, D], f32, kind="ExternalInput")
    w_dram = nc.dram_tensor("w", [D, 2, E], f32, kind="ExternalInput")
    b_dram = nc.dram_tensor("bvec", [BE], f32, kind="ExternalInput")
    out_dram = nc.dram_tensor("out", [BPC, NG, E], f32, kind="ExternalOutput")
    bscratch = nc.dram_tensor("bscratch", [NPAD, BE], f32)
    st_dram = nc.inline_tensor(st_np, name="st_const")
    id_dram = nc.inline_tensor(np.eye(128, dtype=np.float32), name="id_const")
    gi1_dram = nc.inline_tensor(gi1_np, name="gi1_const")
    gi2_dram = nc.inline_tensor(gi2_np, name="gi2_const")

    with tile.TileContext(nc) as tc:
        with (
            tc.tile_pool(name="consts", bufs=1) as consts,
            tc.tile_pool(name="work", bufs=1) as work,
            tc.tile_pool(name="pipe", bufs=3) as pipe,
            tc.tile_pool(name="ps_stage1", bufs=2, space="PSUM") as ps1,
            tc.tile_pool(name="ps_ab", bufs=4, space="PSUM") as psab,
        ):
            # ---- constant / input loads
            ident = consts.tile([128, 128], f32)
            nc.sync.dma_start(out=ident, in_=id_dram[:, :])
            w_sb = consts.tile([128, 2, E], f32)
            nc.sync.dma_start(out=w_sb, in_=w_dram[:, :, :])
            st_sb = consts.tile([NV, NG], f32)
            nc.sync.dma_start(out=st_sb, in_=st_dram[:, :])
            gi1_sb = consts.tile([128, SPLIT // 16], mybir.dt.int16)
            nc.sync.dma_start(out=gi1_sb, in_=gi1_dram[:, :])
            gi2_sb = consts.tile([128, (NIDX - SPLIT) // 16], mybir.dt.int16)
            nc.sync.dma_start(out=gi2_sb, in_=gi2_dram[:, :])
            b2p_sb = consts.tile([NV, BE], f32)
            nc.sync.dma_start(
                out=b2p_sb,
                in_=bass.AP(
                    tensor=b_dram, offset=0, ap=[[0, NV], [1, BE]]
                ),
            )
            x_sb = work.tile([NV, BPC, D], f32)
            nc.sync.dma_start(
                out=x_sb[:, :, :],
                in_=bass.AP(
                    tensor=x_dram,
                    offset=0,
                    ap=[[D, NV], [NV * D, BPC], [1, D]],
                ),
            )

            libload = nc.gpsimd.load_library(library_config.mlp)
            # ---- stage 3a: generate gather descriptors early (prepare_only);
            # Q7 descriptor generation (~8ns/desc) overlaps stages 1-2, and
            # the triggers fire the SDMA transfers once bscratch is written
            g_sb = work.tile([128, GCH, BE], f32)
            sem1 = nc.alloc_semaphore("swdge_g1")
            sem2 = nc.alloc_semaphore("swdge_g2")
            g1 = nc.gpsimd.dma_gather(
                out_ap=g_sb[:, : SPLIT // 128, :],
                in_ap=bscratch[:, :],
                idxs_ap=gi1_sb[:, :],
                num_idxs=SPLIT,
                num_idxs_reg=SPLIT,
                elem_size=BE,
                single_packet=False,
                prepare_only=True,
                sem=sem1,
                queue_num=0,
            )
            g2 = nc.gpsimd.dma_gather(
                out_ap=g_sb[:, SPLIT // 128 :, :],
                in_ap=bscratch[:, :],
                idxs_ap=gi2_sb[:, :],
                num_idxs=NIDX - SPLIT,
                num_idxs_reg=NIDX - SPLIT,
                elem_size=BE,
                single_packet=False,
                prepare_only=True,
                sem=sem2,
                queue_num=1,
            )
            tile.add_dep_helper(g1.ins, libload.ins, reason="prep after lib load")
            tile.add_dep_helper(g2.ins, libload.ins, reason="prep after lib load")

            # ---- stage 1: per batch, transpose vf and project to 32 dims
            p_sb = work.tile([NV, 2, BPC, E], f32)  # (half, batch, e)
            for b in range(BPC):
                xt_ps = ps1.tile([D, NV], f32, tag="xt")
                nc.tensor.transpose(xt_ps, x_sb[:, b, :], ident[:NV, :NV])
                xt_sb = pipe.tile([D, NV], f32, tag="xt_sb")
                nc.vector.tensor_copy(xt_sb, xt_ps)
                pp = ps1.tile([NV, 2, E], f32, tag="pp")
                nc.tensor.matmul(
                    pp.rearrange("p h e -> p (h e)"),
                    lhsT=xt_sb,
                    rhs=w_sb.rearrange("p h e -> p (h e)"),
                    start=True,
                    stop=True,
                )
                nc.scalar.copy(out=p_sb[:, :, b, :], in_=pp)
            # fold b_edge into A: rows of S sum to 1, so S@(P_self + b) = A + b
            nc.vector.tensor_add(p_sb[:, 0, :, :], p_sb[:, 0, :, :], b2p_sb)

            # ---- stage 2: A = S@P_self (+b), B = S@P_nbr, node-major
            a_sb = work.tile([128, NCH, BE], f32)
            b_sb = work.tile([128, NCH, BE], f32)
            for c in range(NCH):
                n0 = 128 * c
                nw = min(128, NG - n0)
                for h, dst in ((0, a_sb), (1, b_sb)):
                    ps = psab.tile([128, BE], f32, tag="ab")
                    nc.tensor.matmul(
                        ps[:nw, :],
                        lhsT=st_sb[:, n0 : n0 + nw],
                        rhs=p_sb[:, h, :, :].rearrange("p b e -> p (b e)"),
                        start=True,
                        stop=True,
                    )
                    nc.vector.tensor_copy(dst[:nw, c, :], ps[:nw, :])
            bdma = nc.sync.dma_start(
                out=bscratch[:, :].rearrange("(c p) e -> p c e", p=128),
                in_=b_sb[:, :, :],
            )

            t1g = nc.gpsimd.trigger_dma(count=None, queue_num=0)
            t2g = nc.gpsimd.trigger_dma(count=None, queue_num=1)
            tile.add_dep_helper(t1g.ins, bdma.ins, reason="trigger after b write")
            tile.add_dep_helper(t2g.ins, bdma.ins, reason="trigger after b write")

            # ---- stage 4: r = relu(A + G); weighted sum over the 14 slots
            r_sb = work.tile([128, GCH, BE], f32)
            for k in range(3):
                nc.vector.tensor_add(
                    r_sb[:, 6 * k : 6 * (k + 1), :],
                    g_sb[:, 6 * k : 6 * (k + 1), :],
                    a_sb,
                )
            a0 = a_sb[:, 0, :]
            a0_b11 = bass.AP(
                tensor=a0.tensor,
                offset=a0.offset,
                ap=[a0.ap[0], [0, DEG - 3], a0.ap[-1]],
            )
            nc.vector.tensor_add(r_sb[:, 18:, :], g_sb[:, 18:, :], a0_b11)
            half = GCH // 2
            nc.scalar.activation(
                r_sb[:, :half, :],
                r_sb[:, :half, :],
                mybir.ActivationFunctionType.Relu,
            )
            nc.vector.tensor_scalar_max(r_sb[:, half:, :], r_sb[:, half:, :], 0.0)

            out_sb = work.tile([128, NCH, BE], f32)
            s_sb = work.tile([128, NCH, BE], f32)
            nc.vector.tensor_add(s_sb, r_sb[:, 0:6, :], r_sb[:, 6:12, :])
            nc.vector.tensor_add(s_sb, s_sb, r_sb[:, 12:18, :])
            # factor rows (chunks 1..5): the third neighbor appears 12x in
            # the padded table -> add 11*r2 more
            nc.vector.scalar_tensor_tensor(
                out_sb[:, 1:, :],
                r_sb[:, 13:18, :],
                11.0,
                s_sb[:, 1:, :],
                op0=mybir.AluOpType.mult,
                op1=mybir.AluOpType.add,
            )
            # chunk 0: rows <120 sum neighbor slots 3..13; rows 120..127 get
            # 11*r2 via the index-table trick (see _host_tables)
            rtail = r_sb[:, 18, :]
            rview = bass.AP(
                tensor=rtail.tensor,
                offset=rtail.offset,
                ap=[rtail.ap[0], [1, BE], [BE, DEG - 3]],
            )
            t3 = work.tile([128, BE], f32)
            nc.vector.tensor_reduce(
                t3, rview, axis=mybir.AxisListType.X, op=mybir.AluOpType.add
            )
            nc.vector.tensor_add(out_sb[:, 0, :], s_sb[:, 0, :], t3)

            # ---- stage 5: store (already node-major, e contiguous per batch)
            for b in range(BPC):
                dst = out_dram[b, 0:640, :].rearrange("(c p) e -> p c e", p=128)
                nc.sync.dma_start(
                    out=dst, in_=out_sb[:, 0:5, E * b : E * (b + 1)]
                )
                nc.sync.dma_start(
                    out=out_dram[b, 640:NG, :],
                    in_=out_sb[0 : NG - 640, 5, E * b : E * (b + 1)],
                )
    # clear the manual prep sems after the tail barrier so the NEFF can be
    # re-executed (sems must start at 0)
    nc.sync.sem_clear(sem1)
    nc.sync.sem_clear(sem2)
    _fix_prep_sems(nc, {"DMASW0": sem1, "DMASW1": sem2})
    _split_multi_waits(nc)
    lower_extended_insts(nc)
    return nc


_NC_CACHE = None


def _get_nc():
    global _NC_CACHE
    if _NC_CACHE is None:
        _NC_CACHE = _build_bass()
    return _NC_CACHE


def _in_maps(node_feats, W_edge, b_edge):
    # fold the /14 edge mean into the MLP weights (relu is pos. homogeneous)
    ws = (W_edge / DEG).reshape(2, D, E).transpose(1, 0, 2)  # [128, 2, 16]
    ws = np.ascontiguousarray(ws.astype(np.float32))
    bvec = np.ascontiguousarray(
        np.tile((b_edge / DEG).astype(np.float32), BPC)
    )  # [(b, e)] = 128
    return [
        {
            "x": np.ascontiguousarray(
                node_feats[c * BPC : (c + 1) * BPC].astype(np.float32)
            ),
            "w": ws,
            "bvec": bvec,
        }
        for c in range(NCORES)
    ]


def kernel(node_feats, W_edge, b_edge):
    node_feats = np.asarray(node_feats, dtype=np.float32)
    W_edge = np.asarray(W_edge, dtype=np.float32)
    b_edge = np.asarray(b_edge, dtype=np.float32)
    assert node_feats.shape == (B, NV, D)

    nc = _get_nc()
    res = run_bass_kernel_spmd(
        nc, _in_maps(node_feats, W_edge, b_edge), core_ids=list(range(NCORES))
    )
    out = np.concatenate([res.results[c]["out"] for c in range(NCORES)], axis=0)
    return out
